# revision 10
# baseline (speedup 1.0000x reference)
"""ALiBi multi-head self-attention on 8 Trainium2 NeuronCores.

Problem: B=2, L=2048, D=1024, H=16, Dh=64, f32 I/O.
  q = X@Wq.T+bq; k = X@Wk.T+bk; v = X@Wv.T+bv   (per-head split)
  S = q k^T/sqrt(Dh) + mask,  mask[h,i,j] = pos_bias[h, i-j+L-1]
  out = softmax(S) v  -> concat heads -> @Wo.T + bo

Sharding: core c -> batch b=c//4, heads [4r, 4r+4) with r=c%4.
Each core computes its 4 heads' attention over its batch and a partial
output projection; a 4-way ReduceScatter per batch-group reduces the
partial (2048,1024) projections, each core emitting a distinct 512-row
slice of the final output.

On-core layout (S^T formulation; keys live on PSUM partitions so the
P@V matmul needs no transposes):
  qT/kT: (Dh on partitions, L on columns), two 128-partition tensors
         per projection; partitions 0-63 = even head, 64-127 = odd head.
  S^T chunk = kT_chunk.T @ qT  (lhsT=kT (64,128keys), rhs=qT (64,512q)),
         the odd head row-packed at tile_position=(64,0).
  P = exp(S^T/8) * exp(mask^T)  -- exp on ScalarE straight out of PSUM,
         mask factor is a host-precomputed sliding-window buffer
         Ebig[p, c] = exp(pb[c - p + 127]); chunk j0 / q-offset q0 uses
         columns [q0 - j0 + 1920, +512).
  O^T += V_chunk_aug.T @ P  with V augmented by a ones column ->
         PSUM row 64 accumulates the softmax denominator for free.
  O^T rows are scaled by 1/denom (DMA-replicated across partitions)
  during PSUM evacuation, then the output projection contracts the
  256 head-dims via 4 row-packed (64,128) matmuls per output tile.

No softmax max-subtraction: |S/8| <= ~4 for any plausible input scale
here, far inside exp's f32/bf16 range. Compute dtype bf16 (PSUM f32).
"""

import sys

sys.path.insert(0, "/opt/trn_rl_repo")

import dataclasses

import ml_dtypes
import numpy as np

import concourse.bass as bass
import concourse.mybir as mybir
import concourse.tile as tile
from concourse import bacc
from concourse.bass_utils import run_bass_kernel_spmd

B, L, D, H, DH = 2, 2048, 1024, 16, 64
NC = 8
HPC = H // 4  # heads per core = 4
HD = HPC * DH  # head dims per core = 256
EW = L + 2048 - 128  # Ebig width = 3968
LQ = 512  # q columns per attention sweep (quarter)
NQ = L // LQ  # 4
KC = 128  # key chunk
NKC = L // KC  # 16
SCALE = 1.0 / np.sqrt(DH)

F32 = mybir.dt.float32
BF16 = mybir.dt.bfloat16

_graph_cache = {}


def _build(shared_mask: bool):
    nc = bacc.Bacc("TRN2", target_bir_lowering=False, debug=False, num_devices=NC)

    xt_d = nc.declare_dram_parameter("xt", [D, L], BF16, isOutput=False)
    wqt_d = nc.declare_dram_parameter("wqt", [D, HD], BF16, isOutput=False)
    wkt_d = nc.declare_dram_parameter("wkt", [D, HD], BF16, isOutput=False)
    wvt_d = nc.declare_dram_parameter("wvt", [D, HD], BF16, isOutput=False)
    wot_d = nc.declare_dram_parameter("wot", [HD, D], BF16, isOutput=False)
    bqk_d = nc.declare_dram_parameter("bqk", [128, 4], F32, isOutput=False)
    bv_d = nc.declare_dram_parameter("bv", [1, HD], F32, isOutput=False)
    bo4_d = nc.declare_dram_parameter("bo4", [1, D], F32, isOutput=False)
    n_ebig = 1 if shared_mask else HPC
    ebig_d = nc.declare_dram_parameter("ebig", [n_ebig, 128, EW], BF16, isOutput=False)
    out_d = nc.declare_dram_parameter("out", [L // 4, D], F32, isOutput=True)

    partial_dram = [nc.dram_tensor(f"partial_{t}", [512, D], BF16) for t in range(NQ)]
    rs_out = [nc.dram_tensor(f"rs_out_{t}", [128, D], BF16) for t in range(NQ)]
    dscr = nc.dram_tensor("dscr", [NQ, 2, 2, 512], F32)  # denom bounce

    with tile.TileContext(nc) as tc:
        with (
            tc.tile_pool(name="const", bufs=1) as cp,
            tc.tile_pool(name="work", bufs=3) as wp,
            tc.tile_pool(name="outp", bufs=2) as op,
            tc.tile_pool(name="psum", bufs=2, space="PSUM") as pp,
        ):
            # ---- stage A: load + projections -------------------------
            xts = []
            for k in range(8):
                t = cp.tile([128, L], BF16, tag=f"xt{k}")
                nc.sync.dma_start(out=t[:, :], in_=xt_d[k * 128 : (k + 1) * 128, :])
                xts.append(t)

            w_sb = {}
            for nm, dten in (("q", wqt_d), ("k", wkt_d), ("v", wvt_d)):
                for k in range(8):
                    t = cp.tile([128, HD], BF16, tag=f"w{nm}{k}")
                    nc.sync.dma_start(out=t[:, :], in_=dten[k * 128 : (k + 1) * 128, :])
                    w_sb[nm, k] = t

            wot_sb = []
            for h in range(HPC):
                t = cp.tile([64, D], BF16, tag=f"wot{h}", name=f"wot{h}")
                nc.sync.dma_start(out=t[:, :], in_=wot_d[h * 64 : (h + 1) * 64, :])
                wot_sb.append(t)

            bqk_sb = cp.tile([128, 4], F32, tag="bqk")
            nc.sync.dma_start(out=bqk_sb[:, :], in_=bqk_d[:, :])
            bv_bc = cp.tile([128, HD], F32, tag="bv_bc")
            nc.gpsimd.dma_start(out=bv_bc[:, :], in_=bv_d[0:1, :].broadcast_to((128, HD)))
            bo4_bc = cp.tile([128, D], F32, tag="bo4_bc")
            nc.gpsimd.dma_start(out=bo4_bc[:, :], in_=bo4_d[0:1, :].broadcast_to((128, D)))

            ebig_sb = []
            for e in range(n_ebig):
                t = cp.tile([128, EW], BF16, tag=f"ebig{e}")
                nc.sync.dma_start(out=t[:, :], in_=ebig_d[e, :, :])
                ebig_sb.append(t)

            def ebig_for(h):
                return ebig_sb[0] if shared_mask else ebig_sb[h]

            # qT / kT: (HD=256 out-dims as 2 partition chunks, L columns)
            qkt_sb = {}
            for nm, bcol0 in (("q", 0), ("k", 2)):
                for pc in range(2):
                    dst = cp.tile([128, L], BF16, tag=f"{nm}T{pc}")
                    qkt_sb[nm, pc] = dst
                    for n2 in range(L // 1024):
                        ps = pp.tile([128, 1024], F32, tag="ps_main")
                        for ng in range(2):
                            for k in range(8):
                                nc.tensor.matmul(
                                    ps[:, ng * 512 : (ng + 1) * 512],
                                    w_sb[nm, k][:, pc * 128 : (pc + 1) * 128],
                                    xts[k][:, n2 * 1024 + ng * 512 : n2 * 1024 + (ng + 1) * 512],
                                    start=(k == 0),
                                    stop=(k == 7),
                                )
                        nc.vector.tensor_scalar_add(
                            dst[:, n2 * 1024 : (n2 + 1) * 1024],
                            ps[:, :],
                            bqk_sb[:, bcol0 + pc : bcol0 + pc + 1],
                        )

            # v: (L on partitions, per (chunk, head) 64 dims + ones col)
            v_all = cp.tile([128, NKC, HPC, DH + 1], BF16, tag="v_all")
            nc.vector.memset(v_all[:, :, :, DH], 1.0)
            for c in range(NKC):
                ps = pp.tile([128, 512], F32, tag="ps_o", bufs=4)
                for k in range(8):
                    nc.tensor.matmul(
                        ps[:, 0:HD],
                        xts[k][:, c * 128 : (c + 1) * 128],
                        w_sb["v", k][:, :],
                        start=(k == 0),
                        stop=(k == 7),
                    )
                nc.vector.tensor_tensor(
                    out=v_all[:, c, :, 0:DH],
                    in0=ps[:, 0:HD].rearrange("p (h d) -> p h d", h=HPC),
                    in1=bv_bc[:, :].rearrange("p (h d) -> p h d", h=HPC),
                    op=mybir.AluOpType.add,
                )

            # per-head O^T accumulators (64 partitions, L columns)
            ot_sb = [cp.tile([64, L], BF16, tag=f"ot{h}", name=f"ot{h}") for h in range(HPC)]
            recipt = wp.tile([128, LQ], F32, tag="recipt")
            rscr = wp.tile([128, LQ], F32, tag="rscr")

            # ---- stage B: attention ----------------------------------
            # The two head-pairs are interleaved at chunk granularity so
            # the TensorEngine always has independent work while ScalarE
            # runs the other pair's exp (keeps PE dense -> HAM stays warm).
            for t in range(NQ):
                q0 = t * LQ
                ops = {
                    (pair, sub): pp.tile(
                        [128, LQ], F32, tag="ps_o", bufs=4, name=f"ops_{t}_{pair}_{sub}"
                    )
                    for pair in range(2)
                    for sub in range(2)
                }
                for j in range(NKC):
                    j0 = j * KC
                    off = q0 - j0 + 1920
                    for pair in range(2):
                        hA, hB = 2 * pair, 2 * pair + 1
                        ps = pp.tile([128, 2 * LQ], F32, tag="ps_main")
                        nc.tensor.matmul(
                            ps[:, 0:LQ],
                            qkt_sb["k", pair][0:64, j0 : j0 + KC],
                            qkt_sb["q", pair][0:64, q0 : q0 + LQ],
                            start=True,
                            stop=True,
                            tile_position=(0, 0),
                        )
                        nc.tensor.matmul(
                            ps[:, LQ : 2 * LQ],
                            qkt_sb["k", pair][64:128, j0 : j0 + KC],
                            qkt_sb["q", pair][64:128, q0 : q0 + LQ],
                            start=True,
                            stop=True,
                            tile_position=(64, 0),
                        )
                        p_sb = wp.tile([128, 2 * LQ], BF16, tag="p_sb")
                        nc.scalar.activation(
                            p_sb[:, :],
                            ps[:, :],
                            mybir.ActivationFunctionType.Exp,
                            scale=float(SCALE),
                        )
                        for hh, sl in ((hA, slice(0, LQ)), (hB, slice(LQ, 2 * LQ))):
                            nc.vector.tensor_tensor(
                                out=p_sb[:, sl],
                                in0=p_sb[:, sl],
                                in1=ebig_for(hh)[:, off : off + LQ],
                                op=mybir.AluOpType.mult,
                            )
                        nc.tensor.matmul(
                            ops[pair, 0][0:65, :],
                            v_all[:, j, hA, 0 : DH + 1],
                            p_sb[:, 0:LQ],
                            start=(j == 0),
                            stop=(j == NKC - 1),
                        )
                        nc.tensor.matmul(
                            ops[pair, 1][0:65, :],
                            v_all[:, j, hB, 0 : DH + 1],
                            p_sb[:, LQ : 2 * LQ],
                            start=(j == 0),
                            stop=(j == NKC - 1),
                        )
                # epilogue: normalize by the accumulated denominators
                for pair in range(2):
                    for sub in range(2):
                        half = slice(0, 64) if sub == 0 else slice(64, 128)
                        dsl = dscr[t, pair, sub, :][None, :]
                        nc.vector.tensor_copy(rscr[64:65, :], ops[pair, sub][64:65, :])
                        nc.sync.dma_start(out=dsl, in_=rscr[64:65, :])
                        nc.gpsimd.dma_start(
                            out=rscr[half, :], in_=dsl.broadcast_to((64, 512))
                        )
                    nc.vector.reciprocal_approx_fast(recipt[:, :], rscr[:, :])
                    for sub in range(2):
                        hh = 2 * pair + sub
                        half = slice(0, 64) if sub == 0 else slice(64, 128)
                        nc.vector.tensor_tensor(
                            out=ot_sb[hh][0:64, q0 : q0 + LQ],
                            in0=ops[pair, sub][0:64, :],
                            in1=recipt[half, :],
                            op=mybir.AluOpType.mult,
                        )
                # output projection for this quarter's 4 q-tiles
                for qi, qt in enumerate(range(t * 4, t * 4 + 4)):
                    out_sb = op.tile([128, D], BF16, tag="out_sb")
                    for n2 in range(2):
                        pso = pp.tile([128, 512], F32, tag="ps_o", bufs=4)
                        for h in range(HPC):
                            nc.tensor.matmul(
                                pso[:, :],
                                ot_sb[h][0:64, qt * 128 : (qt + 1) * 128],
                                wot_sb[h][0:64, n2 * 512 : (n2 + 1) * 512],
                                start=(h == 0),
                                stop=(h == HPC - 1),
                            )
                        nc.vector.tensor_tensor(
                            out=out_sb[:, n2 * 512 : (n2 + 1) * 512],
                            in0=pso[:, :],
                            in1=bo4_bc[:, n2 * 512 : (n2 + 1) * 512],
                            op=mybir.AluOpType.add,
                        )
                    nc.sync.dma_start(
                        out=partial_dram[t][qi * 128 : (qi + 1) * 128, :],
                        in_=out_sb[:, :],
                    )
                # overlapped reduce-scatter of this quarter's partials;
                # rank r of each group receives global q-tile (4t + r)
                nc.gpsimd.collective_compute(
                    "ReduceScatter",
                    mybir.AluOpType.add,
                    replica_groups=[[0, 1, 2, 3], [4, 5, 6, 7]],
                    ins=[partial_dram[t][:, :]],
                    outs=[rs_out[t][:, :]],
                )
                fin_sb = op.tile([128, D], BF16, tag="fin_sb")
                fin32 = op.tile([128, D], F32, tag="fin32")
                nc.sync.dma_start(out=fin_sb[:, :], in_=rs_out[t][:, :])
                nc.vector.tensor_copy(fin32[:, :], fin_sb[:, :])
                nc.sync.dma_start(out=out_d[t * 128 : (t + 1) * 128, :], in_=fin32[:, :])

    nc.compile()
    return nc


def _get_graph(shared_mask: bool):
    key = bool(shared_mask)
    if key not in _graph_cache:
        _graph_cache[key] = _build(key)
    return _graph_cache[key]


def _make_ebig(pb_row: np.ndarray) -> np.ndarray:
    """Ebig[p, c] = exp(pb[c - p + 127]), p in [0,128), c in [0,EW)."""
    idx = (np.arange(EW)[None, :] - np.arange(128)[:, None]) + 127
    return np.exp(pb_row[idx]).astype(ml_dtypes.bfloat16)


def kernel(queries, Wq, bq, Wk, bk, Wv, bv, Wo, bo, pos_bias):
    queries = np.asarray(queries, dtype=np.float32)
    Wq, Wk, Wv, Wo = (np.asarray(w, dtype=np.float32) for w in (Wq, Wk, Wv, Wo))
    bq, bk, bv, bo = (np.asarray(x, dtype=np.float32) for x in (bq, bk, bv, bo))
    pos_bias = np.asarray(pos_bias, dtype=np.float32)

    shared_mask = bool(np.all(pos_bias == pos_bias[0:1]))
    nc = _get_graph(shared_mask)

    xt = [np.ascontiguousarray(queries[b].T).astype(ml_dtypes.bfloat16) for b in range(B)]
    wqt = np.ascontiguousarray(Wq.T).astype(ml_dtypes.bfloat16)
    wkt = np.ascontiguousarray(Wk.T).astype(ml_dtypes.bfloat16)
    wvt = np.ascontiguousarray(Wv.T).astype(ml_dtypes.bfloat16)
    wot = np.ascontiguousarray(Wo.T).astype(ml_dtypes.bfloat16)

    if shared_mask:
        ebig_all = {None: _make_ebig(pos_bias[0])[None]}
    else:
        ebig_all = {h: _make_ebig(pos_bias[h]) for h in range(H)}

    in_maps = []
    for c in range(NC):
        b, r = c // 4, c % 4
        hs = r * HD  # head-dim slice start
        bqs = bq[hs : hs + HD].reshape(2, 128).T
        bks = bk[hs : hs + HD].reshape(2, 128).T
        bqk = np.ascontiguousarray(np.concatenate([bqs, bks], axis=1))
        if shared_mask:
            ebig = ebig_all[None]
        else:
            ebig = np.stack([ebig_all[4 * r + i] for i in range(HPC)])
        in_maps.append(
            {
                "xt": xt[b],
                "wqt": np.ascontiguousarray(wqt[:, hs : hs + HD]),
                "wkt": np.ascontiguousarray(wkt[:, hs : hs + HD]),
                "wvt": np.ascontiguousarray(wvt[:, hs : hs + HD]),
                "wot": np.ascontiguousarray(wot[hs : hs + HD, :]),
                "bqk": bqk,
                "bv": bv[None, hs : hs + HD] * 1.0,
                "bo4": bo[None, :] / 4.0,
                "ebig": np.ascontiguousarray(ebig),
            }
        )

    res = run_bass_kernel_spmd(nc, in_maps, core_ids=list(range(NC)))
    out = np.empty((B, L, D), dtype=np.float32)
    for c in range(NC):
        b, r = c // 4, c % 4
        o = res.results[c]["out"]  # chunk t = global q-tile (4t + r)
        for t in range(4):
            qt = 4 * t + r
            out[b, qt * 128 : (qt + 1) * 128, :] = o[t * 128 : (t + 1) * 128, :]
    return out


# revision 11
# speedup vs baseline: 1.0717x; 1.0717x over previous
"""ALiBi multi-head self-attention on 8 Trainium2 NeuronCores.

Problem: B=2, L=2048, D=1024, H=16, Dh=64, f32 I/O.
  q = X@Wq.T+bq; k = X@Wk.T+bk; v = X@Wv.T+bv   (per-head split)
  S = q k^T/sqrt(Dh) + mask,  mask[h,i,j] = pos_bias[h, i-j+L-1]
  out = softmax(S) v  -> concat heads -> @Wo.T + bo

Sharding: core c -> batch b=c//4, heads [4r, 4r+4) with r=c%4.
Each core computes its 4 heads' attention over its batch and a partial
output projection; a 4-way ReduceScatter per batch-group reduces the
partial (2048,1024) projections, each core emitting a distinct 512-row
slice of the final output.

On-core layout (S^T formulation; keys live on PSUM partitions so the
P@V matmul needs no transposes):
  qT/kT: (Dh on partitions, L on columns), two 128-partition tensors
         per projection; partitions 0-63 = even head, 64-127 = odd head.
  S^T chunk = kT_chunk.T @ qT  (lhsT=kT (64,128keys), rhs=qT (64,512q)),
         the odd head row-packed at tile_position=(64,0).
  P = exp(S^T/8) * exp(mask^T)  -- exp on ScalarE straight out of PSUM,
         mask factor is a host-precomputed sliding-window buffer
         Ebig[p, c] = exp(pb[c - p + 127]); chunk j0 / q-offset q0 uses
         columns [q0 - j0 + 1920, +512).
  O^T += V_chunk_aug.T @ P  with V augmented by a ones column ->
         PSUM row 64 accumulates the softmax denominator for free.
  O^T rows are scaled by 1/denom (DMA-replicated across partitions)
  during PSUM evacuation, then the output projection contracts the
  256 head-dims via 4 row-packed (64,128) matmuls per output tile.

No softmax max-subtraction: |S/8| <= ~4 for any plausible input scale
here, far inside exp's f32/bf16 range. Compute dtype bf16 (PSUM f32).
"""

import sys

sys.path.insert(0, "/opt/trn_rl_repo")

import dataclasses

import ml_dtypes
import numpy as np

import concourse.bass as bass
import concourse.mybir as mybir
import concourse.tile as tile
from concourse import bacc
from concourse.bass_utils import run_bass_kernel_spmd

B, L, D, H, DH = 2, 2048, 1024, 16, 64
NC = 8
HPC = H // 4  # heads per core = 4
HD = HPC * DH  # head dims per core = 256
EW = L + 2048 - 128  # Ebig width = 3968
LQ = 512  # q columns per attention sweep (quarter)
NQ = L // LQ  # 4
KC = 128  # key chunk
NKC = L // KC  # 16
SCALE = 1.0 / np.sqrt(DH)

F32 = mybir.dt.float32
BF16 = mybir.dt.bfloat16

_graph_cache = {}


def _build(shared_mask: bool):
    nc = bacc.Bacc("TRN2", target_bir_lowering=False, debug=False, num_devices=NC)

    xt_d = nc.declare_dram_parameter("xt", [D, L], BF16, isOutput=False)
    wqt_d = nc.declare_dram_parameter("wqt", [D, HD], BF16, isOutput=False)
    wkt_d = nc.declare_dram_parameter("wkt", [D, HD], BF16, isOutput=False)
    wvt_d = nc.declare_dram_parameter("wvt", [D, HD], BF16, isOutput=False)
    wot_d = nc.declare_dram_parameter("wot", [HD, D], BF16, isOutput=False)
    bqk_d = nc.declare_dram_parameter("bqk", [128, 4], F32, isOutput=False)
    bv_d = nc.declare_dram_parameter("bv", [1, HD], F32, isOutput=False)
    bo4_d = nc.declare_dram_parameter("bo4", [1, D], F32, isOutput=False)
    n_ebig = 1 if shared_mask else HPC
    ebig_d = nc.declare_dram_parameter("ebig", [n_ebig, 128, EW], BF16, isOutput=False)
    out_d = nc.declare_dram_parameter("out", [L // 4, D], F32, isOutput=True)

    partial_dram = [nc.dram_tensor(f"partial_{t}", [512, D], BF16) for t in range(NQ)]
    rs_out = [nc.dram_tensor(f"rs_out_{t}", [128, D], BF16) for t in range(NQ)]
    dscr = nc.dram_tensor("dscr", [NQ, 2, 2, 512], F32)  # denom bounce

    with tile.TileContext(nc) as tc:
        with (
            tc.tile_pool(name="const", bufs=1) as cp,
            tc.tile_pool(name="work", bufs=3) as wp,
            tc.tile_pool(name="outp", bufs=2) as op,
            tc.tile_pool(name="psum", bufs=2, space="PSUM") as pp,
        ):
            # ---- stage A: load + projections -------------------------
            xts = []
            for k in range(8):
                t = cp.tile([128, L], BF16, tag=f"xt{k}")
                nc.sync.dma_start(out=t[:, :], in_=xt_d[k * 128 : (k + 1) * 128, :])
                xts.append(t)

            w_sb = {}
            for nm, dten in (("q", wqt_d), ("k", wkt_d), ("v", wvt_d)):
                for k in range(8):
                    t = cp.tile([128, HD], BF16, tag=f"w{nm}{k}")
                    nc.sync.dma_start(out=t[:, :], in_=dten[k * 128 : (k + 1) * 128, :])
                    w_sb[nm, k] = t

            wot_sb = []
            for h in range(HPC):
                t = cp.tile([64, D], BF16, tag=f"wot{h}", name=f"wot{h}")
                nc.sync.dma_start(out=t[:, :], in_=wot_d[h * 64 : (h + 1) * 64, :])
                wot_sb.append(t)

            bqk_sb = cp.tile([128, 4], F32, tag="bqk")
            nc.sync.dma_start(out=bqk_sb[:, :], in_=bqk_d[:, :])
            bv_bc = cp.tile([128, HD], F32, tag="bv_bc")
            nc.gpsimd.dma_start(out=bv_bc[:, :], in_=bv_d[0:1, :].broadcast_to((128, HD)))
            bo4_bc = cp.tile([128, D], F32, tag="bo4_bc")
            nc.gpsimd.dma_start(out=bo4_bc[:, :], in_=bo4_d[0:1, :].broadcast_to((128, D)))

            ebig_sb = []
            for e in range(n_ebig):
                t = cp.tile([128, EW], BF16, tag=f"ebig{e}")
                nc.sync.dma_start(out=t[:, :], in_=ebig_d[e, :, :])
                ebig_sb.append(t)

            def ebig_for(h):
                return ebig_sb[0] if shared_mask else ebig_sb[h]

            # qT / kT: (HD=256 out-dims as 2 partition chunks, L columns)
            qkt_sb = {}
            for nm, bcol0 in (("q", 0), ("k", 2)):
                for pc in range(2):
                    dst = cp.tile([128, L], BF16, tag=f"{nm}T{pc}")
                    qkt_sb[nm, pc] = dst
                    for n2 in range(L // 1024):
                        ps = pp.tile([128, 1024], F32, tag="ps_main")
                        for ng in range(2):
                            for k in range(8):
                                nc.tensor.matmul(
                                    ps[:, ng * 512 : (ng + 1) * 512],
                                    w_sb[nm, k][:, pc * 128 : (pc + 1) * 128],
                                    xts[k][:, n2 * 1024 + ng * 512 : n2 * 1024 + (ng + 1) * 512],
                                    start=(k == 0),
                                    stop=(k == 7),
                                )
                        nc.vector.tensor_scalar_add(
                            dst[:, n2 * 1024 : (n2 + 1) * 1024],
                            ps[:, :],
                            bqk_sb[:, bcol0 + pc : bcol0 + pc + 1],
                        )

            # v: (L on partitions, per (chunk, head) 64 dims + ones col)
            v_all = cp.tile([128, NKC, HPC, DH + 1], BF16, tag="v_all")
            nc.vector.memset(v_all[:, :, :, DH], 1.0)
            for c in range(NKC):
                ps = pp.tile([128, 512], F32, tag="ps_o", bufs=4)
                for k in range(8):
                    nc.tensor.matmul(
                        ps[:, 0:HD],
                        xts[k][:, c * 128 : (c + 1) * 128],
                        w_sb["v", k][:, :],
                        start=(k == 0),
                        stop=(k == 7),
                    )
                nc.vector.tensor_tensor(
                    out=v_all[:, c, :, 0:DH],
                    in0=ps[:, 0:HD].rearrange("p (h d) -> p h d", h=HPC),
                    in1=bv_bc[:, :].rearrange("p (h d) -> p h d", h=HPC),
                    op=mybir.AluOpType.add,
                )

            # per-head O^T accumulators (64 partitions, L columns)
            ot_sb = [cp.tile([64, L], BF16, tag=f"ot{h}", name=f"ot{h}") for h in range(HPC)]
            recipt = wp.tile([128, LQ], F32, tag="recipt")
            rscr = wp.tile([128, LQ], F32, tag="rscr")

            # ---- stage B: attention ----------------------------------
            # The two head-pairs are interleaved at chunk granularity so
            # the TensorEngine always has independent work while ScalarE
            # runs the other pair's exp (keeps PE dense -> HAM stays warm).
            for t in range(NQ):
                q0 = t * LQ
                ops = {
                    (pair, sub): pp.tile(
                        [128, LQ], F32, tag="ps_o", bufs=4, name=f"ops_{t}_{pair}_{sub}"
                    )
                    for pair in range(2)
                    for sub in range(2)
                }
                for j in range(NKC):
                    j0 = j * KC
                    off = q0 - j0 + 1920
                    pstile, ptile = {}, {}
                    for pair in range(2):
                        ps = pp.tile(
                            [128, 2 * LQ], F32, tag="ps_main", name=f"ps_{pair}"
                        )
                        pstile[pair] = ps
                        for sub in range(2):
                            pb = slice(64 * sub, 64 * sub + 64)
                            nc.tensor.matmul(
                                ps[:, sub * LQ : (sub + 1) * LQ],
                                qkt_sb["k", pair][pb, j0 : j0 + KC],
                                qkt_sb["q", pair][pb, q0 : q0 + LQ],
                                start=True,
                                stop=True,
                                tile_position=(64 * sub, 0),
                            )
                    for pair in range(2):
                        p_sb = wp.tile(
                            [128, 2 * LQ], BF16, tag=f"p_sb{pair}", name=f"p_sb{pair}"
                        )
                        ptile[pair] = p_sb
                        nc.scalar.activation(
                            p_sb[:, :],
                            pstile[pair][:, :],
                            mybir.ActivationFunctionType.Exp,
                            scale=float(SCALE),
                        )
                    for pair in range(2):
                        for sub in range(2):
                            nc.vector.tensor_tensor(
                                out=ptile[pair][:, sub * LQ : (sub + 1) * LQ],
                                in0=ptile[pair][:, sub * LQ : (sub + 1) * LQ],
                                in1=ebig_for(2 * pair + sub)[:, off : off + LQ],
                                op=mybir.AluOpType.mult,
                            )
                    for pair in range(2):
                        for sub in range(2):
                            nc.tensor.matmul(
                                ops[pair, sub][0:65, :],
                                v_all[:, j, 2 * pair + sub, 0 : DH + 1],
                                ptile[pair][:, sub * LQ : (sub + 1) * LQ],
                                start=(j == 0),
                                stop=(j == NKC - 1),
                            )
                # epilogue: normalize by the accumulated denominators
                for pair in range(2):
                    for sub in range(2):
                        half = slice(0, 64) if sub == 0 else slice(64, 128)
                        dsl = dscr[t, pair, sub, :][None, :]
                        nc.vector.tensor_copy(rscr[64:65, :], ops[pair, sub][64:65, :])
                        nc.sync.dma_start(out=dsl, in_=rscr[64:65, :])
                        nc.gpsimd.dma_start(
                            out=rscr[half, :], in_=dsl.broadcast_to((64, 512))
                        )
                    nc.vector.reciprocal_approx_fast(recipt[:, :], rscr[:, :])
                    for sub in range(2):
                        hh = 2 * pair + sub
                        half = slice(0, 64) if sub == 0 else slice(64, 128)
                        nc.vector.tensor_tensor(
                            out=ot_sb[hh][0:64, q0 : q0 + LQ],
                            in0=ops[pair, sub][0:64, :],
                            in1=recipt[half, :],
                            op=mybir.AluOpType.mult,
                        )
                # output projection for this quarter's 4 q-tiles
                for qi, qt in enumerate(range(t * 4, t * 4 + 4)):
                    out_sb = op.tile([128, D], BF16, tag="out_sb")
                    for n2 in range(2):
                        pso = pp.tile([128, 512], F32, tag="ps_o", bufs=4)
                        for h in range(HPC):
                            nc.tensor.matmul(
                                pso[:, :],
                                ot_sb[h][0:64, qt * 128 : (qt + 1) * 128],
                                wot_sb[h][0:64, n2 * 512 : (n2 + 1) * 512],
                                start=(h == 0),
                                stop=(h == HPC - 1),
                            )
                        nc.vector.tensor_tensor(
                            out=out_sb[:, n2 * 512 : (n2 + 1) * 512],
                            in0=pso[:, :],
                            in1=bo4_bc[:, n2 * 512 : (n2 + 1) * 512],
                            op=mybir.AluOpType.add,
                        )
                    nc.sync.dma_start(
                        out=partial_dram[t][qi * 128 : (qi + 1) * 128, :],
                        in_=out_sb[:, :],
                    )
                # overlapped reduce-scatter of this quarter's partials;
                # rank r of each group receives global q-tile (4t + r)
                nc.gpsimd.collective_compute(
                    "ReduceScatter",
                    mybir.AluOpType.add,
                    replica_groups=[[0, 1, 2, 3], [4, 5, 6, 7]],
                    ins=[partial_dram[t][:, :]],
                    outs=[rs_out[t][:, :]],
                )
                fin_sb = op.tile([128, D], BF16, tag="fin_sb")
                fin32 = op.tile([128, D], F32, tag="fin32")
                nc.sync.dma_start(out=fin_sb[:, :], in_=rs_out[t][:, :])
                nc.vector.tensor_copy(fin32[:, :], fin_sb[:, :])
                nc.sync.dma_start(out=out_d[t * 128 : (t + 1) * 128, :], in_=fin32[:, :])

    nc.compile()
    return nc


def _get_graph(shared_mask: bool):
    key = bool(shared_mask)
    if key not in _graph_cache:
        _graph_cache[key] = _build(key)
    return _graph_cache[key]


def _make_ebig(pb_row: np.ndarray) -> np.ndarray:
    """Ebig[p, c] = exp(pb[c - p + 127]), p in [0,128), c in [0,EW)."""
    idx = (np.arange(EW)[None, :] - np.arange(128)[:, None]) + 127
    return np.exp(pb_row[idx]).astype(ml_dtypes.bfloat16)


def kernel(queries, Wq, bq, Wk, bk, Wv, bv, Wo, bo, pos_bias):
    queries = np.asarray(queries, dtype=np.float32)
    Wq, Wk, Wv, Wo = (np.asarray(w, dtype=np.float32) for w in (Wq, Wk, Wv, Wo))
    bq, bk, bv, bo = (np.asarray(x, dtype=np.float32) for x in (bq, bk, bv, bo))
    pos_bias = np.asarray(pos_bias, dtype=np.float32)

    shared_mask = bool(np.all(pos_bias == pos_bias[0:1]))
    nc = _get_graph(shared_mask)

    xt = [np.ascontiguousarray(queries[b].T).astype(ml_dtypes.bfloat16) for b in range(B)]
    wqt = np.ascontiguousarray(Wq.T).astype(ml_dtypes.bfloat16)
    wkt = np.ascontiguousarray(Wk.T).astype(ml_dtypes.bfloat16)
    wvt = np.ascontiguousarray(Wv.T).astype(ml_dtypes.bfloat16)
    wot = np.ascontiguousarray(Wo.T).astype(ml_dtypes.bfloat16)

    if shared_mask:
        ebig_all = {None: _make_ebig(pos_bias[0])[None]}
    else:
        ebig_all = {h: _make_ebig(pos_bias[h]) for h in range(H)}

    in_maps = []
    for c in range(NC):
        b, r = c // 4, c % 4
        hs = r * HD  # head-dim slice start
        bqs = bq[hs : hs + HD].reshape(2, 128).T
        bks = bk[hs : hs + HD].reshape(2, 128).T
        bqk = np.ascontiguousarray(np.concatenate([bqs, bks], axis=1))
        if shared_mask:
            ebig = ebig_all[None]
        else:
            ebig = np.stack([ebig_all[4 * r + i] for i in range(HPC)])
        in_maps.append(
            {
                "xt": xt[b],
                "wqt": np.ascontiguousarray(wqt[:, hs : hs + HD]),
                "wkt": np.ascontiguousarray(wkt[:, hs : hs + HD]),
                "wvt": np.ascontiguousarray(wvt[:, hs : hs + HD]),
                "wot": np.ascontiguousarray(wot[hs : hs + HD, :]),
                "bqk": bqk,
                "bv": bv[None, hs : hs + HD] * 1.0,
                "bo4": bo[None, :] / 4.0,
                "ebig": np.ascontiguousarray(ebig),
            }
        )

    res = run_bass_kernel_spmd(nc, in_maps, core_ids=list(range(NC)))
    out = np.empty((B, L, D), dtype=np.float32)
    for c in range(NC):
        b, r = c // 4, c % 4
        o = res.results[c]["out"]  # chunk t = global q-tile (4t + r)
        for t in range(4):
            qt = 4 * t + r
            out[b, qt * 128 : (qt + 1) * 128, :] = o[t * 128 : (t + 1) * 128, :]
    return out


# revision 14
# speedup vs baseline: 1.1241x; 1.0489x over previous
"""ALiBi multi-head self-attention on 8 Trainium2 NeuronCores.

Problem: B=2, L=2048, D=1024, H=16, Dh=64, f32 I/O.
  q = X@Wq.T+bq; k = X@Wk.T+bk; v = X@Wv.T+bv   (per-head split)
  S = q k^T/sqrt(Dh) + mask,  mask[h,i,j] = pos_bias[h, i-j+L-1]
  out = softmax(S) v  -> concat heads -> @Wo.T + bo

Sharding: core c -> batch b=c//4, heads [4r, 4r+4) with r=c%4.
Each core computes its 4 heads' attention over its batch and a partial
output projection; a 4-way ReduceScatter per batch-group reduces the
partial (2048,1024) projections, each core emitting a distinct 512-row
slice of the final output.

On-core layout (S^T formulation; keys live on PSUM partitions so the
P@V matmul needs no transposes):
  qT/kT: (Dh on partitions, L on columns), two 128-partition tensors
         per projection; partitions 0-63 = even head, 64-127 = odd head.
  S^T chunk = kT_chunk.T @ qT  (lhsT=kT (64,128keys), rhs=qT (64,512q)),
         the odd head row-packed at tile_position=(64,0).
  P = exp(S^T/8) * exp(mask^T)  -- exp on ScalarE straight out of PSUM,
         mask factor is a host-precomputed sliding-window buffer
         Ebig[p, c] = exp(pb[c - p + 127]); chunk j0 / q-offset q0 uses
         columns [q0 - j0 + 1920, +512).
  O^T += V_chunk_aug.T @ P  with V augmented by a ones column ->
         PSUM row 64 accumulates the softmax denominator for free.
  O^T rows are scaled by 1/denom (DMA-replicated across partitions)
  during PSUM evacuation, then the output projection contracts the
  256 head-dims via 4 row-packed (64,128) matmuls per output tile.

No softmax max-subtraction: |S/8| <= ~4 for any plausible input scale
here, far inside exp's f32/bf16 range. Compute dtype bf16 (PSUM f32).
"""

import sys

sys.path.insert(0, "/opt/trn_rl_repo")

import dataclasses

import ml_dtypes
import numpy as np

import concourse.bass as bass
import concourse.mybir as mybir
import concourse.tile as tile
from concourse import bacc
from concourse.bass_utils import run_bass_kernel_spmd

B, L, D, H, DH = 2, 2048, 1024, 16, 64
NC = 8
HPC = H // 4  # heads per core = 4
HD = HPC * DH  # head dims per core = 256
EW = L + 2048 - 128  # Ebig width = 3968
LQ = 512  # q columns per attention sweep (quarter)
NQ = L // LQ  # 4
KC = 128  # key chunk
NKC = L // KC  # 16
SCALE = 1.0 / np.sqrt(DH)

F32 = mybir.dt.float32
BF16 = mybir.dt.bfloat16

_graph_cache = {}


def _build(shared_mask: bool):
    nc = bacc.Bacc("TRN2", target_bir_lowering=False, debug=False, num_devices=NC)

    xt_d = nc.declare_dram_parameter("xt", [D, L], BF16, isOutput=False)
    wqt_d = nc.declare_dram_parameter("wqt", [D, HD], BF16, isOutput=False)
    wkt_d = nc.declare_dram_parameter("wkt", [D, HD], BF16, isOutput=False)
    wvt_d = nc.declare_dram_parameter("wvt", [D, HD], BF16, isOutput=False)
    wot_d = nc.declare_dram_parameter("wot", [HD, D], BF16, isOutput=False)
    bqk_d = nc.declare_dram_parameter("bqk", [128, 4], F32, isOutput=False)
    bv_d = nc.declare_dram_parameter("bv", [1, HD], F32, isOutput=False)
    bo4_d = nc.declare_dram_parameter("bo4", [1, D], F32, isOutput=False)
    n_ebig = 1 if shared_mask else HPC
    ebig_d = nc.declare_dram_parameter("ebig", [n_ebig, 128, EW], BF16, isOutput=False)
    out_d = nc.declare_dram_parameter("out", [L // 4, D], F32, isOutput=True)

    partial_dram = [nc.dram_tensor(f"partial_{t}", [512, D], BF16) for t in range(NQ)]
    rs_out = [nc.dram_tensor(f"rs_out_{t}", [128, D], BF16) for t in range(NQ)]
    dscr = nc.dram_tensor("dscr", [NQ, 2, 2, 512], F32)  # denom bounce

    with tile.TileContext(nc) as tc:
        with (
            tc.tile_pool(name="const", bufs=1) as cp,
            tc.tile_pool(name="work", bufs=3) as wp,
            tc.tile_pool(name="outp", bufs=2) as op,
            tc.tile_pool(name="psum", bufs=2, space="PSUM") as pp,
        ):
            # ---- stage A: load + projections -------------------------
            xts = []
            w_sb = {}
            for k in range(8):
                t = cp.tile([128, L], BF16, tag=f"xt{k}", name=f"xt{k}")
                nc.sync.dma_start(out=t[:, :], in_=xt_d[k * 128 : (k + 1) * 128, :])
                xts.append(t)
                for nm, dten in (("q", wqt_d), ("k", wkt_d), ("v", wvt_d)):
                    w = cp.tile([128, HD], BF16, tag=f"w{nm}{k}", name=f"w{nm}{k}")
                    nc.sync.dma_start(out=w[:, :], in_=dten[k * 128 : (k + 1) * 128, :])
                    w_sb[nm, k] = w

            wot_sb = []
            for h in range(HPC):
                t = cp.tile([64, D], BF16, tag=f"wot{h}", name=f"wot{h}")
                nc.sync.dma_start(out=t[:, :], in_=wot_d[h * 64 : (h + 1) * 64, :])
                wot_sb.append(t)

            bqk_sb = cp.tile([128, 4], F32, tag="bqk")
            nc.sync.dma_start(out=bqk_sb[:, :], in_=bqk_d[:, :])
            bv_bc = cp.tile([128, HD], F32, tag="bv_bc")
            nc.gpsimd.dma_start(out=bv_bc[:, :], in_=bv_d[0:1, :].broadcast_to((128, HD)))
            bo4_bc = cp.tile([128, D], F32, tag="bo4_bc")
            nc.gpsimd.dma_start(out=bo4_bc[:, :], in_=bo4_d[0:1, :].broadcast_to((128, D)))

            ebig_sb = []
            for e in range(n_ebig):
                t = cp.tile([128, EW], BF16, tag=f"ebig{e}")
                nc.sync.dma_start(out=t[:, :], in_=ebig_d[e, :, :])
                ebig_sb.append(t)

            def ebig_for(h):
                return ebig_sb[0] if shared_mask else ebig_sb[h]

            # qT / kT: (HD=256 out-dims as 2 partition chunks, L columns)
            qkt_sb = {}
            for nm, bcol0 in (("q", 0), ("k", 2)):
                for pc in range(2):
                    dst = cp.tile([128, L], BF16, tag=f"{nm}T{pc}")
                    qkt_sb[nm, pc] = dst
                    for n2 in range(L // 1024):
                        ps = pp.tile([128, 1024], F32, tag="ps_main")
                        for ng in range(2):
                            for k in range(8):
                                nc.tensor.matmul(
                                    ps[:, ng * 512 : (ng + 1) * 512],
                                    w_sb[nm, k][:, pc * 128 : (pc + 1) * 128],
                                    xts[k][:, n2 * 1024 + ng * 512 : n2 * 1024 + (ng + 1) * 512],
                                    start=(k == 0),
                                    stop=(k == 7),
                                )
                        nc.vector.tensor_scalar_add(
                            dst[:, n2 * 1024 : (n2 + 1) * 1024],
                            ps[:, :],
                            bqk_sb[:, bcol0 + pc : bcol0 + pc + 1],
                        )

            # v: (L on partitions, per (chunk, head) 64 dims + ones col)
            v_all = cp.tile([128, NKC, HPC, DH + 1], BF16, tag="v_all")
            nc.vector.memset(v_all[:, :, :, DH], 1.0)
            for c in range(NKC):
                ps = pp.tile([128, 512], F32, tag="ps_o", bufs=4)
                for k in range(8):
                    nc.tensor.matmul(
                        ps[:, 0:HD],
                        xts[k][:, c * 128 : (c + 1) * 128],
                        w_sb["v", k][:, :],
                        start=(k == 0),
                        stop=(k == 7),
                    )
                nc.vector.tensor_tensor(
                    out=v_all[:, c, :, 0:DH],
                    in0=ps[:, 0:HD].rearrange("p (h d) -> p h d", h=HPC),
                    in1=bv_bc[:, :].rearrange("p (h d) -> p h d", h=HPC),
                    op=mybir.AluOpType.add,
                )

            # per-head O^T accumulators (64 partitions, L columns)
            ot_sb = [cp.tile([64, L], BF16, tag=f"ot{h}", name=f"ot{h}") for h in range(HPC)]
            recipt = wp.tile([128, LQ], F32, tag="recipt")
            rscr = wp.tile([128, LQ], F32, tag="rscr")

            def emit_oproj(tt):
                # projection of quarter tt's 4 q-tiles + overlapped
                # reduce-scatter; rank r receives global q-tile (4tt + r)
                for qi, qt in enumerate(range(tt * 4, tt * 4 + 4)):
                    out_sb = op.tile([128, D], BF16, tag="out_sb", name="out_sb")
                    pso = pp.tile([128, 1024], F32, tag="ps_main", name="pso")
                    for n2 in range(2):
                        for h in range(HPC):
                            nc.tensor.matmul(
                                pso[:, n2 * 512 : (n2 + 1) * 512],
                                ot_sb[h][0:64, qt * 128 : (qt + 1) * 128],
                                wot_sb[h][0:64, n2 * 512 : (n2 + 1) * 512],
                                start=(h == 0),
                                stop=(h == HPC - 1),
                            )
                    nc.vector.tensor_tensor(
                        out=out_sb[:, :],
                        in0=pso[:, :],
                        in1=bo4_bc[:, :],
                        op=mybir.AluOpType.add,
                    )
                    nc.sync.dma_start(
                        out=partial_dram[tt][qi * 128 : (qi + 1) * 128, :],
                        in_=out_sb[:, :],
                    )
                nc.gpsimd.collective_compute(
                    "ReduceScatter",
                    mybir.AluOpType.add,
                    replica_groups=[[0, 1, 2, 3], [4, 5, 6, 7]],
                    ins=[partial_dram[tt][:, :]],
                    outs=[rs_out[tt][:, :]],
                )
                fin_sb = op.tile([128, D], BF16, tag="fin_sb", name="fin_sb")
                fin32 = op.tile([128, D], F32, tag="fin32", name="fin32")
                nc.sync.dma_start(out=fin_sb[:, :], in_=rs_out[tt][:, :])
                nc.vector.tensor_copy(fin32[:, :], fin_sb[:, :])
                nc.sync.dma_start(
                    out=out_d[tt * 128 : (tt + 1) * 128, :], in_=fin32[:, :]
                )

            # ---- stage B: attention ----------------------------------
            # The two head-pairs are interleaved at chunk granularity so
            # the TensorEngine always has independent work while ScalarE
            # runs the other pair's exp (keeps PE dense -> HAM stays warm).
            for t in range(NQ):
                q0 = t * LQ
                ops = {
                    (pair, sub): pp.tile(
                        [128, LQ], F32, tag="ps_o", bufs=4, name=f"ops_{t}_{pair}_{sub}"
                    )
                    for pair in range(2)
                    for sub in range(2)
                }
                for j in range(NKC):
                    j0 = j * KC
                    off = q0 - j0 + 1920
                    pstile, ptile = {}, {}
                    for pair in range(2):
                        ps = pp.tile(
                            [128, 2 * LQ], F32, tag="ps_main", name=f"ps_{pair}"
                        )
                        pstile[pair] = ps
                        for sub in range(2):
                            pb = slice(64 * sub, 64 * sub + 64)
                            nc.tensor.matmul(
                                ps[:, sub * LQ : (sub + 1) * LQ],
                                qkt_sb["k", pair][pb, j0 : j0 + KC],
                                qkt_sb["q", pair][pb, q0 : q0 + LQ],
                                start=True,
                                stop=True,
                                tile_position=(64 * sub, 0),
                            )
                    for pair in range(2):
                        p_sb = wp.tile(
                            [128, 2 * LQ], BF16, tag=f"p_sb{pair}", name=f"p_sb{pair}", bufs=5
                        )
                        ptile[pair] = p_sb
                        nc.scalar.activation(
                            p_sb[:, :],
                            pstile[pair][:, :],
                            mybir.ActivationFunctionType.Exp,
                            scale=float(SCALE),
                        )
                    for pair in range(2):
                        for sub in range(2):
                            nc.vector.tensor_tensor(
                                out=ptile[pair][:, sub * LQ : (sub + 1) * LQ],
                                in0=ptile[pair][:, sub * LQ : (sub + 1) * LQ],
                                in1=ebig_for(2 * pair + sub)[:, off : off + LQ],
                                op=mybir.AluOpType.mult,
                            )
                    for pair in range(2):
                        for sub in range(2):
                            nc.tensor.matmul(
                                ops[pair, sub][0:65, :],
                                v_all[:, j, 2 * pair + sub, 0 : DH + 1],
                                ptile[pair][:, sub * LQ : (sub + 1) * LQ],
                                start=(j == 0),
                                stop=(j == NKC - 1),
                            )
                # epilogue: normalize by the accumulated denominators
                for pair in range(2):
                    for sub in range(2):
                        half = slice(0, 64) if sub == 0 else slice(64, 128)
                        dsl = dscr[t, pair, sub, :][None, :]
                        nc.vector.tensor_copy(rscr[64:65, :], ops[pair, sub][64:65, :])
                        nc.sync.dma_start(out=dsl, in_=rscr[64:65, :])
                        nc.gpsimd.dma_start(
                            out=rscr[half, :], in_=dsl.broadcast_to((64, 512))
                        )
                    nc.vector.reciprocal_approx_fast(recipt[:, :], rscr[:, :])
                    for sub in range(2):
                        hh = 2 * pair + sub
                        half = slice(0, 64) if sub == 0 else slice(64, 128)
                        nc.vector.tensor_tensor(
                            out=ot_sb[hh][0:64, q0 : q0 + LQ],
                            in0=ops[pair, sub][0:64, :],
                            in1=recipt[half, :],
                            op=mybir.AluOpType.mult,
                        )
                # output projection for quarter tt runs a quarter late so
                # it fills the boundary while tt+1's denominators bounce
                # through DRAM (PE stream: [chunks t] [oproj t-1] [chunks t+1]).
                if t >= 1:
                    emit_oproj(t - 1)
            emit_oproj(NQ - 1)

    nc.compile()
    return nc


def _get_graph(shared_mask: bool):
    key = bool(shared_mask)
    if key not in _graph_cache:
        _graph_cache[key] = _build(key)
    return _graph_cache[key]


def _make_ebig(pb_row: np.ndarray) -> np.ndarray:
    """Ebig[p, c] = exp(pb[c - p + 127]), p in [0,128), c in [0,EW)."""
    idx = (np.arange(EW)[None, :] - np.arange(128)[:, None]) + 127
    return np.exp(pb_row[idx]).astype(ml_dtypes.bfloat16)


def kernel(queries, Wq, bq, Wk, bk, Wv, bv, Wo, bo, pos_bias):
    queries = np.asarray(queries, dtype=np.float32)
    Wq, Wk, Wv, Wo = (np.asarray(w, dtype=np.float32) for w in (Wq, Wk, Wv, Wo))
    bq, bk, bv, bo = (np.asarray(x, dtype=np.float32) for x in (bq, bk, bv, bo))
    pos_bias = np.asarray(pos_bias, dtype=np.float32)

    shared_mask = bool(np.all(pos_bias == pos_bias[0:1]))
    nc = _get_graph(shared_mask)

    xt = [np.ascontiguousarray(queries[b].T).astype(ml_dtypes.bfloat16) for b in range(B)]
    wqt = np.ascontiguousarray(Wq.T).astype(ml_dtypes.bfloat16)
    wkt = np.ascontiguousarray(Wk.T).astype(ml_dtypes.bfloat16)
    wvt = np.ascontiguousarray(Wv.T).astype(ml_dtypes.bfloat16)
    wot = np.ascontiguousarray(Wo.T).astype(ml_dtypes.bfloat16)

    if shared_mask:
        ebig_all = {None: _make_ebig(pos_bias[0])[None]}
    else:
        ebig_all = {h: _make_ebig(pos_bias[h]) for h in range(H)}

    in_maps = []
    for c in range(NC):
        b, r = c // 4, c % 4
        hs = r * HD  # head-dim slice start
        bqs = bq[hs : hs + HD].reshape(2, 128).T
        bks = bk[hs : hs + HD].reshape(2, 128).T
        bqk = np.ascontiguousarray(np.concatenate([bqs, bks], axis=1))
        if shared_mask:
            ebig = ebig_all[None]
        else:
            ebig = np.stack([ebig_all[4 * r + i] for i in range(HPC)])
        in_maps.append(
            {
                "xt": xt[b],
                "wqt": np.ascontiguousarray(wqt[:, hs : hs + HD]),
                "wkt": np.ascontiguousarray(wkt[:, hs : hs + HD]),
                "wvt": np.ascontiguousarray(wvt[:, hs : hs + HD]),
                "wot": np.ascontiguousarray(wot[hs : hs + HD, :]),
                "bqk": bqk,
                "bv": bv[None, hs : hs + HD] * 1.0,
                "bo4": bo[None, :] / 4.0,
                "ebig": np.ascontiguousarray(ebig),
            }
        )

    res = run_bass_kernel_spmd(nc, in_maps, core_ids=list(range(NC)))
    out = np.empty((B, L, D), dtype=np.float32)
    for c in range(NC):
        b, r = c // 4, c % 4
        o = res.results[c]["out"]  # chunk t = global q-tile (4t + r)
        for t in range(4):
            qt = 4 * t + r
            out[b, qt * 128 : (qt + 1) * 128, :] = o[t * 128 : (t + 1) * 128, :]
    return out


# revision 19
# speedup vs baseline: 1.2441x; 1.1067x over previous
"""ALiBi multi-head self-attention on 8 Trainium2 NeuronCores.

Problem: B=2, L=2048, D=1024, H=16, Dh=64, f32 I/O.
  q = X@Wq.T+bq; k = X@Wk.T+bk; v = X@Wv.T+bv   (per-head split)
  S = q k^T/sqrt(Dh) + mask,  mask[h,i,j] = pos_bias[h, i-j+L-1]
  out = softmax(S) v  -> concat heads -> @Wo.T + bo

Sharding: core c -> batch b=c//4, heads [4r, 4r+4) with r=c%4.
Each core computes its 4 heads' attention over its batch and a partial
output projection; a 4-way ReduceScatter per batch-group reduces the
partial (2048,1024) projections, each core emitting a distinct 512-row
slice of the final output.

On-core layout (S^T formulation; keys live on PSUM partitions so the
P@V matmul needs no transposes):
  qT/kT: (Dh on partitions, L on columns), two 128-partition tensors
         per projection; partitions 0-63 = even head, 64-127 = odd head.
  S^T chunk = kT_chunk.T @ qT  (lhsT=kT (64,128keys), rhs=qT (64,512q)),
         the odd head row-packed at tile_position=(64,0).
  P = exp(S^T/8) * exp(mask^T)  -- exp on ScalarE straight out of PSUM,
         mask factor is a host-precomputed sliding-window buffer
         Ebig[p, c] = exp(pb[c - p + 127]); chunk j0 / q-offset q0 uses
         columns [q0 - j0 + 1920, +512).
  O^T += V_chunk_aug.T @ P  with V augmented by a ones column ->
         PSUM row 64 accumulates the softmax denominator for free.
  O^T rows are scaled by 1/denom (DMA-replicated across partitions)
  during PSUM evacuation, then the output projection contracts the
  256 head-dims via 4 row-packed (64,128) matmuls per output tile.

No softmax max-subtraction: |S/8| <= ~4 for any plausible input scale
here, far inside exp's f32/bf16 range. Compute dtype bf16 (PSUM f32).
"""

import sys

sys.path.insert(0, "/opt/trn_rl_repo")

import dataclasses

import ml_dtypes
import numpy as np

import concourse.bass as bass
import concourse.mybir as mybir
import concourse.tile as tile
from concourse import bacc
from concourse.bass_utils import run_bass_kernel_spmd

B, L, D, H, DH = 2, 2048, 1024, 16, 64
NC = 8
HPC = H // 4  # heads per core = 4
HD = HPC * DH  # head dims per core = 256
EW = L + 2048 - 128  # Ebig width = 3968
LQ = 512  # q columns per attention sweep (quarter)
NQ = L // LQ  # 4
KC = 128  # key chunk
NKC = L // KC  # 16
SCALE = 1.0 / np.sqrt(DH)

F32 = mybir.dt.float32
BF16 = mybir.dt.bfloat16

_graph_cache = {}


def _build(shared_mask: bool):
    nc = bacc.Bacc("TRN2", target_bir_lowering=False, debug=False, num_devices=NC)

    xt_d = nc.declare_dram_parameter("xt", [D, L], BF16, isOutput=False)
    wqt_d = nc.declare_dram_parameter("wqt", [D, HD], BF16, isOutput=False)
    wkt_d = nc.declare_dram_parameter("wkt", [D, HD], BF16, isOutput=False)
    wvt_d = nc.declare_dram_parameter("wvt", [D, HD], BF16, isOutput=False)
    wot_d = nc.declare_dram_parameter("wot", [HD, D], BF16, isOutput=False)
    bqk_d = nc.declare_dram_parameter("bqk", [128, 4], F32, isOutput=False)
    bv_d = nc.declare_dram_parameter("bv", [1, HD], F32, isOutput=False)
    bo4_d = nc.declare_dram_parameter("bo4", [1, D], F32, isOutput=False)
    n_ebig = 1 if shared_mask else HPC
    ebig_d = nc.declare_dram_parameter("ebig", [n_ebig, 128, EW], BF16, isOutput=False)
    out_d = nc.declare_dram_parameter("out", [L // 4, D], F32, isOutput=True)

    partial_dram = [nc.dram_tensor(f"partial_{t}", [512, D], BF16) for t in range(NQ)]
    dscr = nc.dram_tensor("dscr", [NQ, 2, 2 * LQ], F32)  # denom bounce
    rs_out = [nc.dram_tensor(f"rs_out_{t}", [128, D], BF16) for t in range(NQ)]

    with tile.TileContext(nc) as tc:
        with (
            tc.tile_pool(name="const", bufs=1) as cp,
            tc.tile_pool(name="work", bufs=3) as wp,
            tc.tile_pool(name="outp", bufs=2) as op,
            tc.tile_pool(name="psum", bufs=2, space="PSUM") as pp,
        ):
            # ---- stage A: load + projections -------------------------
            xts = []
            w_sb = {}
            for k in range(8):
                t = cp.tile([128, L], BF16, tag=f"xt{k}", name=f"xt{k}")
                nc.sync.dma_start(out=t[:, :], in_=xt_d[k * 128 : (k + 1) * 128, :])
                xts.append(t)
                for nm, dten in (("q", wqt_d), ("k", wkt_d), ("v", wvt_d)):
                    w = cp.tile([128, HD], BF16, tag=f"w{nm}{k}", name=f"w{nm}{k}")
                    nc.sync.dma_start(out=w[:, :], in_=dten[k * 128 : (k + 1) * 128, :])
                    w_sb[nm, k] = w

            wot_sb = []
            for h in range(HPC):
                t = cp.tile([64, D], BF16, tag=f"wot{h}", name=f"wot{h}")
                nc.sync.dma_start(out=t[:, :], in_=wot_d[h * 64 : (h + 1) * 64, :])
                wot_sb.append(t)

            bqk_sb = cp.tile([128, 4], F32, tag="bqk")
            nc.sync.dma_start(out=bqk_sb[:, :], in_=bqk_d[:, :])
            bv_bc = cp.tile([128, HD], F32, tag="bv_bc")
            nc.gpsimd.dma_start(out=bv_bc[:, :], in_=bv_d[0:1, :].broadcast_to((128, HD)))
            bo4_bc = cp.tile([128, D], F32, tag="bo4_bc")
            nc.gpsimd.dma_start(out=bo4_bc[:, :], in_=bo4_d[0:1, :].broadcast_to((128, D)))

            ebig_sb = []
            for e in range(n_ebig):
                t = cp.tile([128, EW], BF16, tag=f"ebig{e}")
                nc.sync.dma_start(out=t[:, :], in_=ebig_d[e, :, :])
                ebig_sb.append(t)

            def ebig_for(h):
                return ebig_sb[0] if shared_mask else ebig_sb[h]

            # qT / kT: (HD=256 out-dims as 2 partition chunks, L columns)
            qkt_sb = {}
            for nm, bcol0 in (("q", 0), ("k", 2)):
                for pc in range(2):
                    dst = cp.tile([128, L], BF16, tag=f"{nm}T{pc}")
                    qkt_sb[nm, pc] = dst
                    for n2 in range(L // 1024):
                        ps = pp.tile([128, 1024], F32, tag="ps_main")
                        for ng in range(2):
                            for k in range(8):
                                nc.tensor.matmul(
                                    ps[:, ng * 512 : (ng + 1) * 512],
                                    w_sb[nm, k][:, pc * 128 : (pc + 1) * 128],
                                    xts[k][:, n2 * 1024 + ng * 512 : n2 * 1024 + (ng + 1) * 512],
                                    start=(k == 0),
                                    stop=(k == 7),
                                )
                        nc.vector.tensor_scalar_add(
                            dst[:, n2 * 1024 : (n2 + 1) * 1024],
                            ps[:, :],
                            bqk_sb[:, bcol0 + pc : bcol0 + pc + 1],
                        )

            # v: (L on partitions, per (chunk, head) 64 dims + ones col)
            v_all = cp.tile([128, NKC, HPC, DH + 1], BF16, tag="v_all")
            nc.vector.memset(v_all[:, :, :, DH], 1.0)
            for c in range(NKC):
                ps = pp.tile([128, 512], F32, tag="ps_o", bufs=4)
                for k in range(8):
                    nc.tensor.matmul(
                        ps[:, 0:HD],
                        xts[k][:, c * 128 : (c + 1) * 128],
                        w_sb["v", k][:, :],
                        start=(k == 0),
                        stop=(k == 7),
                    )
                nc.vector.tensor_tensor(
                    out=v_all[:, c, :, 0:DH],
                    in0=ps[:, 0:HD].rearrange("p (h d) -> p h d", h=HPC),
                    in1=bv_bc[:, :].rearrange("p (h d) -> p h d", h=HPC),
                    op=mybir.AluOpType.add,
                )

            # per-head O^T accumulators (64 partitions, L columns)
            ot_sb = [cp.tile([64, L], BF16, tag=f"ot{h}", name=f"ot{h}") for h in range(HPC)]
            recipt = wp.tile([128, 2 * LQ], F32, tag="recipt", bufs=2)
            rscr = wp.tile([128, 2 * LQ], F32, tag="rscr", bufs=2)
            rbc = wp.tile([128, 2 * LQ], F32, tag="rbc", bufs=2)

            def emit_oproj(tt):
                # projection of quarter tt's 4 q-tiles + overlapped
                # reduce-scatter; rank r receives global q-tile (4tt + r)
                for qi, qt in enumerate(range(tt * 4, tt * 4 + 4)):
                    out_sb = op.tile([128, D], BF16, tag="out_sb", name="out_sb")
                    pso = pp.tile([128, 1024], F32, tag="ps_main", name="pso")
                    for n2 in range(2):
                        for h in range(HPC):
                            nc.tensor.matmul(
                                pso[:, n2 * 512 : (n2 + 1) * 512],
                                ot_sb[h][0:64, qt * 128 : (qt + 1) * 128],
                                wot_sb[h][0:64, n2 * 512 : (n2 + 1) * 512],
                                start=(h == 0),
                                stop=(h == HPC - 1),
                            )
                    nc.vector.tensor_tensor(
                        out=out_sb[:, :],
                        in0=pso[:, :],
                        in1=bo4_bc[:, :],
                        op=mybir.AluOpType.add,
                    )
                    nc.sync.dma_start(
                        out=partial_dram[tt][qi * 128 : (qi + 1) * 128, :],
                        in_=out_sb[:, :],
                    )
                nc.gpsimd.collective_compute(
                    "ReduceScatter",
                    mybir.AluOpType.add,
                    replica_groups=[[0, 1, 2, 3], [4, 5, 6, 7]],
                    ins=[partial_dram[tt][:, :]],
                    outs=[rs_out[tt][:, :]],
                )
                fin_sb = op.tile([128, D], BF16, tag="fin_sb", name="fin_sb")
                fin32 = op.tile([128, D], F32, tag="fin32", name="fin32")
                nc.sync.dma_start(out=fin_sb[:, :], in_=rs_out[tt][:, :])
                nc.vector.tensor_copy(fin32[:, :], fin_sb[:, :])
                nc.sync.dma_start(
                    out=out_d[tt * 128 : (tt + 1) * 128, :], in_=fin32[:, :]
                )

            # ---- stage B: attention ----------------------------------
            # The two head-pairs are interleaved at chunk granularity so
            # the TensorEngine always has independent work while ScalarE
            # runs the other pair's exp (keeps PE dense -> HAM stays warm).
            for t in range(NQ):
                q0 = t * LQ
                ops = {
                    (pair, sub): pp.tile(
                        [128, LQ], F32, tag="ps_o", bufs=4, name=f"ops_{t}_{pair}_{sub}"
                    )
                    for pair in range(2)
                    for sub in range(2)
                }
                for j in range(NKC):
                    j0 = j * KC
                    off = q0 - j0 + 1920
                    pstile, ptile = {}, {}
                    for pair in range(2):
                        ps = pp.tile(
                            [128, 2 * LQ], F32, tag="ps_main", name=f"ps_{pair}"
                        )
                        pstile[pair] = ps
                        for sub in range(2):
                            pb = slice(64 * sub, 64 * sub + 64)
                            nc.tensor.matmul(
                                ps[:, sub * LQ : (sub + 1) * LQ],
                                qkt_sb["k", pair][pb, j0 : j0 + KC],
                                qkt_sb["q", pair][pb, q0 : q0 + LQ],
                                start=True,
                                stop=True,
                                tile_position=(64 * sub, 0),
                            )
                    for pair in range(2):
                        p_sb = wp.tile(
                            [128, 2 * LQ], BF16, tag=f"p_sb{pair}", name=f"p_sb{pair}", bufs=5
                        )
                        ptile[pair] = p_sb
                        nc.scalar.activation(
                            p_sb[:, :],
                            pstile[pair][:, :],
                            mybir.ActivationFunctionType.Exp,
                            scale=float(SCALE),
                        )
                    for pair in range(2):
                        for sub in range(2):
                            nc.vector.tensor_tensor(
                                out=ptile[pair][:, sub * LQ : (sub + 1) * LQ],
                                in0=ptile[pair][:, sub * LQ : (sub + 1) * LQ],
                                in1=ebig_for(2 * pair + sub)[:, off : off + LQ],
                                op=mybir.AluOpType.mult,
                            )
                    for pair in range(2):
                        for sub in range(2):
                            nc.tensor.matmul(
                                ops[pair, sub][0:65, :],
                                v_all[:, j, 2 * pair + sub, 0 : DH + 1],
                                ptile[pair][:, sub * LQ : (sub + 1) * LQ],
                                start=(j == 0),
                                stop=(j == NKC - 1),
                            )
                # epilogue: evacuate O^T unnormalized (frees the PSUM
                # accumulators immediately so the next quarter starts),
                # bounce the denominator rows DRAM->broadcast in the
                # background, and scale OT in place lazily -- the output
                # projection that reads OT is deferred a full quarter.
                for pair in range(2):
                    rs_ = wp.tile([128, 2 * LQ], F32, tag="rscr", bufs=2, name="rs_")
                    rb_ = wp.tile([128, 2 * LQ], F32, tag="rbc", bufs=2, name="rb_")
                    rc_ = wp.tile([128, 2 * LQ], F32, tag="recipt", bufs=2, name="rc_")
                    for sub in range(2):
                        hh = 2 * pair + sub
                        nc.vector.tensor_copy(
                            rs_[64:65, sub * LQ : (sub + 1) * LQ],
                            ops[pair, sub][64:65, :],
                        )
                        nc.vector.tensor_copy(
                            ot_sb[hh][0:64, q0 : q0 + LQ], ops[pair, sub][0:64, :]
                        )
                    dsl = dscr[t, pair, :][None, :]
                    nc.sync.dma_start(out=dsl, in_=rs_[64:65, :])
                    nc.gpsimd.dma_start(
                        out=rb_[:, :], in_=dsl.broadcast_to((128, 2 * LQ))
                    )
                    nc.vector.reciprocal_approx_fast(rc_[:, :], rb_[:, :])
                    for sub in range(2):
                        hh = 2 * pair + sub
                        nc.vector.tensor_tensor(
                            out=ot_sb[hh][0:64, q0 : q0 + LQ],
                            in0=ot_sb[hh][0:64, q0 : q0 + LQ],
                            in1=rc_[0:64, sub * LQ : (sub + 1) * LQ],
                            op=mybir.AluOpType.mult,
                        )
                # output projection for quarter tt runs a quarter late so
                # it fills the boundary while tt+1's denominators bounce
                # through DRAM (PE stream: [chunks t] [oproj t-1] [chunks t+1]).
                if t >= 1:
                    emit_oproj(t - 1)
            emit_oproj(NQ - 1)

    nc.compile()
    return nc


def _get_graph(shared_mask: bool):
    key = bool(shared_mask)
    if key not in _graph_cache:
        _graph_cache[key] = _build(key)
    return _graph_cache[key]


def _make_ebig(pb_row: np.ndarray) -> np.ndarray:
    """Ebig[p, c] = exp(pb[c - p + 127]), p in [0,128), c in [0,EW)."""
    idx = (np.arange(EW)[None, :] - np.arange(128)[:, None]) + 127
    return np.exp(pb_row[idx]).astype(ml_dtypes.bfloat16)


def kernel(queries, Wq, bq, Wk, bk, Wv, bv, Wo, bo, pos_bias):
    queries = np.asarray(queries, dtype=np.float32)
    Wq, Wk, Wv, Wo = (np.asarray(w, dtype=np.float32) for w in (Wq, Wk, Wv, Wo))
    bq, bk, bv, bo = (np.asarray(x, dtype=np.float32) for x in (bq, bk, bv, bo))
    pos_bias = np.asarray(pos_bias, dtype=np.float32)

    shared_mask = bool(np.all(pos_bias == pos_bias[0:1]))
    nc = _get_graph(shared_mask)

    xt = [np.ascontiguousarray(queries[b].T).astype(ml_dtypes.bfloat16) for b in range(B)]
    wqt = np.ascontiguousarray(Wq.T).astype(ml_dtypes.bfloat16)
    wkt = np.ascontiguousarray(Wk.T).astype(ml_dtypes.bfloat16)
    wvt = np.ascontiguousarray(Wv.T).astype(ml_dtypes.bfloat16)
    wot = np.ascontiguousarray(Wo.T).astype(ml_dtypes.bfloat16)

    if shared_mask:
        ebig_all = {None: _make_ebig(pos_bias[0])[None]}
    else:
        ebig_all = {h: _make_ebig(pos_bias[h]) for h in range(H)}

    in_maps = []
    for c in range(NC):
        b, r = c // 4, c % 4
        hs = r * HD  # head-dim slice start
        bqs = bq[hs : hs + HD].reshape(2, 128).T
        bks = bk[hs : hs + HD].reshape(2, 128).T
        bqk = np.ascontiguousarray(np.concatenate([bqs, bks], axis=1))
        if shared_mask:
            ebig = ebig_all[None]
        else:
            ebig = np.stack([ebig_all[4 * r + i] for i in range(HPC)])
        in_maps.append(
            {
                "xt": xt[b],
                "wqt": np.ascontiguousarray(wqt[:, hs : hs + HD]),
                "wkt": np.ascontiguousarray(wkt[:, hs : hs + HD]),
                "wvt": np.ascontiguousarray(wvt[:, hs : hs + HD]),
                "wot": np.ascontiguousarray(wot[hs : hs + HD, :]),
                "bqk": bqk,
                "bv": bv[None, hs : hs + HD] * 1.0,
                "bo4": bo[None, :] / 4.0,
                "ebig": np.ascontiguousarray(ebig),
            }
        )

    res = run_bass_kernel_spmd(nc, in_maps, core_ids=list(range(NC)))
    out = np.empty((B, L, D), dtype=np.float32)
    for c in range(NC):
        b, r = c // 4, c % 4
        o = res.results[c]["out"]  # chunk t = global q-tile (4t + r)
        for t in range(4):
            qt = 4 * t + r
            out[b, qt * 128 : (qt + 1) * 128, :] = o[t * 128 : (t + 1) * 128, :]
    return out
